# revision 51
# baseline (speedup 1.0000x reference)
"""Trainium2 Bass kernel: MechanicsPINN residual (MLP field + biharmonic stencil).

Math (reference): f = MLP(x_coloc) -> [B, H*W]; residual = L(L(f)) + L(f) + f - P
where L is the 5-point reflect-padded Laplacian (EI = KC = GC = 1, dx = dy = 1).

Key transform: the stencil operator A = L^2 + L + I is linear and acts on the
pixel axis, and f is linear in W4, so A(f) = h3 @ A(W4) + A(b4). A(W4) is
precomputed on the host (input-independent weight prep), which removes every
stencil op and halo row from the device program:

    residual = h3 @ W4' - (P - A(b4)),   W4' = A(W4)

Sharding: tensor-parallel over the 65536 output pixels; core c owns columns
[8192c, 8192c+8192) of W4' (no halos needed). On device, the 8192 pixels are
split into two 4096-px halves stacked on the partition axis (partitions 0-63 =
batch for half A, 64-127 = batch for half B) via PE column tiling, so the big
matmul uses all 128 PE columns with B=64.

Dtypes: W4' is streamed as fp8 e3m4 (x4 scale; the 1/4 is folded into W3 via
relu positive-homogeneity, so no device-side dequant). P as e3m4 (x2 scale,
folded into the PSUM evacuation). Output bf16, upcast on host. This halves the
dominant HBM stream (W4') vs bf16; measured end-to-end rel err ~1.5e-2 < 2e-2.

Schedule: the kernel is input-bandwidth-bound (~10.2 MB/core in at ~420 GB/s
across both HWDGE rings). The MLP relu+bias runs on DVE (tensor_scalar), so
both issuing engines are pure DMA queues. In-flight DMAs on a ring complete
with lag proportional to their own size (packet round-robin), so the
h3-critical MLP weights travel as small pieces split across both rings, and
W4' as 16 half-chunks consumed in arrival order, all resident in SBUF. Dummy
matmuls fill the PE's data-wait windows to keep the HAM clock gate at 8/8.
"""

import numpy as np
import ml_dtypes

import concourse.bass as bass
import concourse.tile as tile
from concourse import bacc, mybir
from concourse.bass_utils import run_bass_kernel_spmd

F32 = mybir.dt.float32
BF16 = mybir.dt.bfloat16
FP8 = mybir.dt.float8e3
BF16_NP = ml_dtypes.bfloat16
FP8_NP = ml_dtypes.float8_e3m4

B = 64          # batch (collocation samples)
H = 256
W = 256
NCORES = 8
PIX = 8192      # pixels per core
HALF = 4096     # pixels per partition-half
CW = 512        # matmul column chunk width
CP = 8          # column chunks per half
KT = 8          # k tiles of the 1024-dim contraction
SW = 4.0        # W4' fp8 scale (1/SW folded into W3)
SP = 2.0        # P fp8 scale

_PROGRAM_CACHE = {}


def _build_program():
    nc = bacc.Bacc("TRN2", target_bir_lowering=False, debug=False)

    XW1 = nc.declare_dram_parameter("XW1", [2, 320], F32, isOutput=False)
    bias = nc.declare_dram_parameter("bias", [128, 14], F32, isOutput=False)
    W2M = nc.declare_dram_parameter("W2M", [128, 1024], BF16, isOutput=False)
    W3Q = nc.declare_dram_parameter("W3Q", [128, 4, 1024], BF16, isOutput=False)
    W4q = nc.declare_dram_parameter("W4q", [128, CP, 2, KT, CW], FP8, isOutput=False)
    Pm = nc.declare_dram_parameter("Pm", [128, HALF], FP8, isOutput=False)
    out = nc.declare_dram_parameter("out", [CP, 128, CW], BF16, isOutput=True)

    MUL = mybir.AluOpType.mult
    ADD = mybir.AluOpType.add
    MAX = mybir.AluOpType.max

    with tile.TileContext(nc) as tc:
        with (
            tc.tile_pool(name="singles", bufs=1) as singles,
            tc.tile_pool(name="wpool", bufs=1) as wpool,
            tc.tile_pool(name="rpool", bufs=CP) as rpool,
        ):
            dma = nc.sync.dma_start
            TS = nc.vector.tensor_scalar

            XW1_sb = singles.tile([2, 320], F32)
            bias_sb = singles.tile([128, 14], F32)
            W2M_sb = singles.tile([128, 1024], BF16)
            W3Q_sb = singles.tile([128, 4, 1024], BF16)
            h1_sb = singles.tile([128, 2, B], BF16)
            h2_sb = singles.tile([128, 4, B], BF16)
            h3_sb = singles.tile([128, KT, B], BF16)
            Pm_sb = singles.tile([128, HALF], FP8)
            wts = []
            for j in range(CP):
                wts.append(
                    wpool.tile([128, 2, KT, CW], FP8, tag=f"wt{j}", name=f"wt{j}")
                )

            # Both HWDGE rings are pure DMA queues (the MLP relu runs on DVE,
            # so neither issuing engine has compute). In-flight DMAs on a ring
            # complete with a lag proportional to their OWN size (packet-level
            # round-robin), so the critical MLP weights travel as three small
            # pieces split across both rings, and W4' as 0.5MB half-chunks.
            dma(out=XW1_sb[:, :], in_=XW1[:, :])
            dma(out=bias_sb[:, :], in_=bias[:, :])
            dma(out=W2M_sb[:, :], in_=W2M[:, :])
            dma(out=W3Q_sb[:, 0], in_=W3Q[:, 0])
            dma(out=W3Q_sb[:, 1], in_=W3Q[:, 1])
            nc.scalar.dma_start(out=W3Q_sb[:, 2], in_=W3Q[:, 2])
            nc.scalar.dma_start(out=W3Q_sb[:, 3], in_=W3Q[:, 3])
            nc.scalar.dma_start(out=Pm_sb[:, :], in_=Pm[:, :])
            for j in range(CP):
                eng = dma if j < 4 else nc.scalar.dma_start
                for h in range(2):
                    eng(out=wts[j][:, h], in_=W4q[:, j, h])

            # ---- MLP (transposed activations: h_T[feat, batch]); relu+bias
            # as one DVE tensor_scalar: max(psum + b, 0) ----
            # W2 slice [128,128]: col = k*512 + m*128; W3: col = 1024 + k*1024 + m*128
            with tc.tile_pool(name="mlp_psum", bufs=1, space="PSUM") as mp:
                # HAM warm-up: dummy matmuls fill the PE's natural wait windows
                # (boot->XW1, h1->WM arrival) so the clock gate is at 8/8 for
                # the real work; they never delay it (the waits are data-gated)
                scratch = singles.tile([128, 512], BF16)
                nc.vector.memset(scratch, 0.0)
                wps = mp.tile([64, 512], F32, tag="warm")
                # per-chunk PSUM regions in ALTERNATING tile pairs: Tile's WAR
                # tracking is tile-granular, so alternating breaks the
                # MM-group -> TS -> MM-group serialization between chunks
                ps1 = [mp.tile([128, B], F32, tag="ps1a", name="ps1a"),
                       mp.tile([128, B], F32, tag="ps1b", name="ps1b")]
                ps2 = [mp.tile([128, 2, B], F32, tag="ps2a", name="ps2a"),
                       mp.tile([128, 2, B], F32, tag="ps2b", name="ps2b")]
                ps3 = [mp.tile([128, 4, B], F32, tag="ps3a", name="ps3a"),
                       mp.tile([128, 4, B], F32, tag="ps3b", name="ps3b")]

                def warm(n, cols=64):
                    # dummies keep the HAM clock gate busy; narrow ones (64
                    # cols) never block data-ready work for long, wide ones
                    # (512) force sustained busy during known data waits
                    for _ in range(n):
                        nc.tensor.matmul(
                            wps[:, 0:cols] if cols < 512 else wps,
                            scratch[:, 0:64], scratch[:, 0:cols],
                            start=True, stop=True,
                        )

                warm(14)
                for m in range(2):
                    ps = ps1[m % 2]
                    nc.tensor.matmul(
                        ps, XW1_sb[:, 64 + m * 128 : 64 + (m + 1) * 128],
                        XW1_sb[:, 0:64],
                        start=True, stop=True,
                    )
                    TS(out=h1_sb[:, m, :], in0=ps, scalar1=bias_sb[:, m : m + 1],
                       scalar2=0.0, op0=ADD, op1=MAX)
                warm(8)  # keep HAM warm while the h2 matmuls wait for W2
                for m in range(4):
                    ps = ps2[m % 2][:, m // 2, :]
                    for k in range(2):
                        c0 = k * 512 + m * 128
                        nc.tensor.matmul(
                            ps, W2M_sb[:, c0 : c0 + 128], h1_sb[:, k, :],
                            start=(k == 0), stop=(k == 1),
                        )
                    TS(out=h2_sb[:, m, :], in0=ps, scalar1=bias_sb[:, 2 + m : 3 + m],
                       scalar2=0.0, op0=ADD, op1=MAX)
                    warm(1)
                warm(6)  # bridge the wait for the W3 eighths
                for m in range(8):
                    ps = ps3[m % 2][:, m // 2, :]
                    for k in range(4):
                        nc.tensor.matmul(
                            ps, W3Q_sb[:, k, m * 128 : (m + 1) * 128], h2_sb[:, k, :],
                            start=(k == 0), stop=(k == 3),
                        )
                    TS(out=h3_sb[:, m, :], in0=ps, scalar1=bias_sb[:, 6 + m : 7 + m],
                       scalar2=0.0, op0=ADD, op1=MAX)
                    warm(1)  # keep the HAM busy window dense through h3
                warm(4)  # bridge any wait for the first W4' piece

            # ---- main matmul: half A -> partitions 0-63, half B -> partitions
            # 64-127, both into the SAME PSUM columns (disjoint partitions, one
            # bank per chunk); the two PE column groups run concurrently.
            # Chunks consumed in DMA arrival order (SP leads after the gate) ----
            STT = nc.vector.scalar_tensor_tensor
            with tc.tile_pool(name="ppool", bufs=6, space="PSUM") as ppool:
                wps2 = ppool.tile([64, 64], F32, tag="warm2", bufs=1)

                def warm2(n):
                    # the main loop is delivery-bound (~2.4us/chunk arrival vs
                    # ~1.7us PE); narrow dummies absorb the per-chunk waits so
                    # the HAM clock gate never re-throttles mid-loop
                    for _ in range(n):
                        nc.tensor.matmul(
                            wps2, scratch[:, 0:64], scratch[:, 0:64],
                            start=True, stop=True,
                        )

                for n, i in enumerate((0, 4, 1, 5, 2, 6, 3, 7)):
                    wt = wts[i]
                    ps = ppool.tile([128, CW], F32)
                    for k in range(KT):
                        nc.tensor.matmul(
                            ps[0:64, :], h3_sb[:, k, :], wt[:, 0, k, :],
                            start=(k == 0), stop=(k == KT - 1), tile_position=(0, 0),
                        )
                        nc.tensor.matmul(
                            ps[64:128, :], h3_sb[:, k, :], wt[:, 1, k, :],
                            start=(k == 0), stop=(k == KT - 1), tile_position=(0, 64),
                        )
                    # residual = psum - Pm/SP, written bf16
                    rt = rpool.tile([128, CW], BF16, tag="rt")
                    cb = i * CW
                    STT(out=rt[:, :], in0=Pm_sb[:, cb : cb + CW],
                        scalar=-1.0 / SP, in1=ps[:, :], op0=MUL, op1=ADD)
                    # out stores split 5/3 across rings to balance total bytes
                    oeng = dma if n < 5 else nc.scalar.dma_start
                    oeng(out=out[i], in_=rt[:, :])
                    if n < 7:
                        warm2(6 if n == 6 else 2)

    nc.compile()
    return nc


def _lap(x):
    # reflect-pad width-1 Laplacian on the last two axes (dx = dy = 1)
    p = np.pad(x, [(0, 0)] * (x.ndim - 2) + [(1, 1), (0, 0)], mode="reflect")
    d2y = p[..., :-2, :] - 2.0 * x + p[..., 2:, :]
    p = np.pad(x, [(0, 0)] * (x.ndim - 2) + [(0, 0), (1, 1)], mode="reflect")
    d2x = p[..., :-2] - 2.0 * x + p[..., 2:]
    return d2x + d2y


def make_in_maps(inputs):
    f32 = np.float32
    # offline weight prep: fold the stencil operator into W4/b4
    W4i = np.asarray(inputs["W4"], dtype=f32).reshape(1024, H, W)
    L1 = _lap(W4i)
    W4p = (_lap(L1) + L1 + W4i).reshape(1024, H * W)
    b4i = np.asarray(inputs["b4"], dtype=f32).reshape(H, W)
    l1 = _lap(b4i)
    b4p = (_lap(l1) + l1 + b4i).reshape(H * W)

    W4q_all = np.clip(W4p * SW, -15.5, 15.5).astype(FP8_NP)  # [1024, 65536]

    W2t = np.asarray(inputs["W2"], dtype=f32).reshape(2, 128, 512).transpose(1, 0, 2).reshape(128, 1024)
    # 1/SW folded into W3 (exact: power-of-two scale, relu-homogeneous)
    W3t = (np.asarray(inputs["W3"], dtype=f32) / SW).reshape(4, 128, 1024).transpose(1, 0, 2).reshape(128, 4096)
    shared = {
        "XW1": np.ascontiguousarray(
            np.concatenate([inputs["x_coloc"].T, inputs["W1"]], axis=1), dtype=f32
        ),
        "W2M": np.ascontiguousarray(W2t.astype(BF16_NP)),
        "W3Q": np.ascontiguousarray(W3t.reshape(128, 4, 1024).astype(BF16_NP)),
        "bias": np.ascontiguousarray(
            np.concatenate(
                [
                    np.asarray(inputs["b1"], dtype=f32).reshape(2, 128).T,
                    np.asarray(inputs["b2"], dtype=f32).reshape(4, 128).T,
                    (np.asarray(inputs["b3"], dtype=f32) / SW).reshape(8, 128).T,
                ],
                axis=1,
            )
        ),
    }

    Pme = (np.asarray(inputs["P"], dtype=f32) - b4p[None, :]) * SP  # [B, 65536]
    in_maps = []
    for c in range(NCORES):
        c0 = c * PIX
        # [kt, kp, half, cp, px] -> [kp, cp, half, kt, px]
        Wc = W4q_all[:, c0 : c0 + PIX].reshape(KT, 128, 2, CP, CW).transpose(1, 3, 2, 0, 4)
        Pc = Pme[:, c0 : c0 + PIX].reshape(B, 2, HALF)
        Pc = np.concatenate([Pc[:, 0, :], Pc[:, 1, :]], axis=0)  # [128, HALF]
        m = dict(shared)
        m["W4q"] = np.ascontiguousarray(Wc)
        m["Pm"] = np.clip(Pc, -15.5, 15.5).astype(FP8_NP)
        in_maps.append(m)
    return in_maps


def assemble_output(results):
    outf = np.empty((B, H * W), dtype=np.float32)
    for c in range(NCORES):
        oc = np.asarray(results[c]["out"])  # [CP, 128, CW] bf16
        # [cp, half*64+b, px] -> [b, half, cp, px]
        blk = oc.reshape(CP, 2, B, CW).transpose(2, 1, 0, 3).reshape(B, PIX)
        outf[:, c * PIX : (c + 1) * PIX] = blk.astype(np.float32)
    return outf


def get_program():
    if "nc" not in _PROGRAM_CACHE:
        _PROGRAM_CACHE["nc"] = _build_program()
    return _PROGRAM_CACHE["nc"]


def kernel(**inputs):
    nc = get_program()
    in_maps = make_in_maps(inputs)
    res = run_bass_kernel_spmd(nc, in_maps, list(range(NCORES)))
    return assemble_output(res.results)


# revision 52
# speedup vs baseline: 1.0885x; 1.0885x over previous
"""Trainium2 Bass kernel: MechanicsPINN residual (MLP field + biharmonic stencil).

Math (reference): f = MLP(x_coloc) -> [B, H*W]; residual = L(L(f)) + L(f) + f - P
where L is the 5-point reflect-padded Laplacian (EI = KC = GC = 1, dx = dy = 1).

Key transform: the stencil operator A = L^2 + L + I is linear and acts on the
pixel axis, and f is linear in W4, so A(f) = h3 @ A(W4) + A(b4). A(W4) is
precomputed on the host (input-independent weight prep), which removes every
stencil op and halo row from the device program:

    residual = h3 @ W4' - (P - A(b4)),   W4' = A(W4)

Sharding: tensor-parallel over the 65536 output pixels; core c owns columns
[8192c, 8192c+8192) of W4' (no halos needed). On device, the 8192 pixels are
split into two 4096-px halves stacked on the partition axis (partitions 0-63 =
batch for half A, 64-127 = batch for half B) via PE column tiling, so the big
matmul uses all 128 PE columns with B=64.

Dtypes: W4' is streamed as fp8 e3m4 (x4 scale; the 1/4 is folded into W3 via
relu positive-homogeneity, so no device-side dequant). P as e3m4 (x2 scale,
folded into the PSUM evacuation). Output bf16, upcast on host. This halves the
dominant HBM stream (W4') vs bf16; measured end-to-end rel err ~1.5e-2 < 2e-2.

Schedule: the kernel is input-bandwidth-bound (~10.2 MB/core in at ~420 GB/s
across both HWDGE rings). The MLP relu+bias runs on DVE (tensor_scalar), so
both issuing engines are pure DMA queues. In-flight DMAs on a ring complete
with lag proportional to their own size (packet round-robin), so the
h3-critical MLP weights travel as small pieces split across both rings, and
W4' as 16 half-chunks consumed in arrival order, all resident in SBUF. Dummy
matmuls fill the PE's data-wait windows to keep the HAM clock gate at 8/8.
"""

import numpy as np
import ml_dtypes

import concourse.bass as bass
import concourse.tile as tile
from concourse import bacc, mybir
from concourse.bass_utils import run_bass_kernel_spmd

F32 = mybir.dt.float32
BF16 = mybir.dt.bfloat16
FP8 = mybir.dt.float8e3
BF16_NP = ml_dtypes.bfloat16
FP8_NP = ml_dtypes.float8_e3m4

B = 64          # batch (collocation samples)
H = 256
W = 256
NCORES = 8
PIX = 8192      # pixels per core
HALF = 4096     # pixels per partition-half
CW = 512        # matmul column chunk width
CP = 8          # column chunks per half
KT = 8          # k tiles of the 1024-dim contraction
SW = 4.0        # W4' fp8 scale (1/SW folded into W3)
SP = 2.0        # P fp8 scale

_PROGRAM_CACHE = {}


def _build_program():
    nc = bacc.Bacc("TRN2", target_bir_lowering=False, debug=False)

    XW1 = nc.declare_dram_parameter("XW1", [2, 320], F32, isOutput=False)
    bias = nc.declare_dram_parameter("bias", [128, 14], F32, isOutput=False)
    W2M = nc.declare_dram_parameter("W2M", [128, 1024], BF16, isOutput=False)
    W3Q = nc.declare_dram_parameter("W3Q", [128, 4, 1024], BF16, isOutput=False)
    W4q = nc.declare_dram_parameter("W4q", [128, CP, 2, KT, CW], FP8, isOutput=False)
    Pm = nc.declare_dram_parameter("Pm", [128, HALF], FP8, isOutput=False)
    out = nc.declare_dram_parameter("out", [CP, 128, CW], BF16, isOutput=True)

    MUL = mybir.AluOpType.mult
    ADD = mybir.AluOpType.add
    MAX = mybir.AluOpType.max

    with tile.TileContext(nc) as tc:
        with (
            tc.tile_pool(name="singles", bufs=1) as singles,
            tc.tile_pool(name="wpool", bufs=1) as wpool,
            tc.tile_pool(name="rpool", bufs=CP) as rpool,
        ):
            dma = nc.sync.dma_start
            TS = nc.vector.tensor_scalar

            XW1_sb = singles.tile([2, 320], F32)
            bias_sb = singles.tile([128, 14], F32)
            W2M_sb = singles.tile([128, 1024], BF16)
            W3Q_sb = singles.tile([128, 4, 1024], BF16)
            h1_sb = singles.tile([128, 2, B], BF16)
            h2_sb = singles.tile([128, 4, B], BF16)
            h3_sb = singles.tile([128, KT, B], BF16)
            Pm_sb = singles.tile([128, HALF], FP8)
            wts = []
            for j in range(CP):
                wts.append(
                    wpool.tile([128, 2, KT, CW], FP8, tag=f"wt{j}", name=f"wt{j}")
                )

            # Both HWDGE rings are pure DMA queues (the MLP relu runs on DVE,
            # so neither issuing engine has compute). In-flight DMAs on a ring
            # complete with a lag proportional to their OWN size (packet-level
            # round-robin), so the critical MLP weights travel as three small
            # pieces split across both rings, and W4' as 0.5MB half-chunks.
            dma(out=XW1_sb[:, :], in_=XW1[:, :])
            dma(out=bias_sb[:, :], in_=bias[:, :])
            dma(out=W2M_sb[:, 0:512], in_=W2M[:, 0:512])
            nc.scalar.dma_start(out=W2M_sb[:, 512:1024], in_=W2M[:, 512:1024])
            # W3 in eighths, first-needed halves (h3 chunks 0-3 read cols
            # 0:512 of every k) leading on both rings
            for h in range(2):
                for k in range(4):
                    eng = dma if (k + h) % 2 == 0 else nc.scalar.dma_start
                    eng(out=W3Q_sb[:, k, h * 512 : (h + 1) * 512],
                        in_=W3Q[:, k, h * 512 : (h + 1) * 512])
            nc.scalar.dma_start(out=Pm_sb[:, :], in_=Pm[:, :])
            for j in range(CP):
                eng = dma if j < 4 else nc.scalar.dma_start
                for h in range(2):
                    if j in (3, 7):
                        # last-consumed chunks: k-split quarters so the final
                        # arrivals enable the k-loop sooner
                        eng(out=wts[j][:, h, 0:4], in_=W4q[:, j, h, 0:4])
                        eng(out=wts[j][:, h, 4:8], in_=W4q[:, j, h, 4:8])
                    else:
                        eng(out=wts[j][:, h], in_=W4q[:, j, h])

            # ---- MLP (transposed activations: h_T[feat, batch]); relu+bias
            # as one DVE tensor_scalar: max(psum + b, 0) ----
            # W2 slice [128,128]: col = k*512 + m*128; W3: col = 1024 + k*1024 + m*128
            with tc.tile_pool(name="mlp_psum", bufs=1, space="PSUM") as mp:
                # HAM warm-up: dummy matmuls fill the PE's natural wait windows
                # (boot->XW1, h1->WM arrival) so the clock gate is at 8/8 for
                # the real work; they never delay it (the waits are data-gated)
                scratch = singles.tile([128, 512], BF16)
                nc.vector.memset(scratch, 0.0)
                wps = mp.tile([64, 512], F32, tag="warm")
                # per-chunk PSUM regions in ALTERNATING tile pairs: Tile's WAR
                # tracking is tile-granular, so alternating breaks the
                # MM-group -> TS -> MM-group serialization between chunks
                ps1 = [mp.tile([128, B], F32, tag="ps1a", name="ps1a"),
                       mp.tile([128, B], F32, tag="ps1b", name="ps1b")]
                ps2 = [mp.tile([128, 2, B], F32, tag="ps2a", name="ps2a"),
                       mp.tile([128, 2, B], F32, tag="ps2b", name="ps2b")]
                ps3 = [mp.tile([128, 4, B], F32, tag="ps3a", name="ps3a"),
                       mp.tile([128, 4, B], F32, tag="ps3b", name="ps3b")]

                def warm(n, cols=64):
                    # dummies keep the HAM clock gate busy; narrow ones (64
                    # cols) never block data-ready work for long, wide ones
                    # (512) force sustained busy during known data waits
                    for _ in range(n):
                        nc.tensor.matmul(
                            wps[:, 0:cols] if cols < 512 else wps,
                            scratch[:, 0:64], scratch[:, 0:cols],
                            start=True, stop=True,
                        )

                warm(14)
                for m in range(2):
                    ps = ps1[m % 2]
                    nc.tensor.matmul(
                        ps, XW1_sb[:, 64 + m * 128 : 64 + (m + 1) * 128],
                        XW1_sb[:, 0:64],
                        start=True, stop=True,
                    )
                    TS(out=h1_sb[:, m, :], in0=ps, scalar1=bias_sb[:, m : m + 1],
                       scalar2=0.0, op0=ADD, op1=MAX)
                warm(8)  # keep HAM warm while the h2 matmuls wait for W2
                for m in range(4):
                    ps = ps2[m % 2][:, m // 2, :]
                    for k in range(2):
                        c0 = k * 512 + m * 128
                        nc.tensor.matmul(
                            ps, W2M_sb[:, c0 : c0 + 128], h1_sb[:, k, :],
                            start=(k == 0), stop=(k == 1),
                        )
                    TS(out=h2_sb[:, m, :], in0=ps, scalar1=bias_sb[:, 2 + m : 3 + m],
                       scalar2=0.0, op0=ADD, op1=MAX)
                    warm(1)
                warm(6)  # bridge the wait for the W3 eighths
                for m in range(8):
                    ps = ps3[m % 2][:, m // 2, :]
                    for k in range(4):
                        nc.tensor.matmul(
                            ps, W3Q_sb[:, k, m * 128 : (m + 1) * 128], h2_sb[:, k, :],
                            start=(k == 0), stop=(k == 3),
                        )
                    TS(out=h3_sb[:, m, :], in0=ps, scalar1=bias_sb[:, 6 + m : 7 + m],
                       scalar2=0.0, op0=ADD, op1=MAX)
                    warm(1)  # keep the HAM busy window dense through h3
                warm(4)  # bridge any wait for the first W4' piece

            # ---- main matmul: half A -> partitions 0-63, half B -> partitions
            # 64-127, both into the SAME PSUM columns (disjoint partitions, one
            # bank per chunk); the two PE column groups run concurrently.
            # Chunks consumed in DMA arrival order (SP leads after the gate) ----
            STT = nc.vector.scalar_tensor_tensor
            with tc.tile_pool(name="ppool", bufs=6, space="PSUM") as ppool:
                wps2 = ppool.tile([64, 64], F32, tag="warm2", bufs=1)

                def warm2(n):
                    # the main loop is delivery-bound (~2.4us/chunk arrival vs
                    # ~1.7us PE); narrow dummies absorb the per-chunk waits so
                    # the HAM clock gate never re-throttles mid-loop
                    for _ in range(n):
                        nc.tensor.matmul(
                            wps2, scratch[:, 0:64], scratch[:, 0:64],
                            start=True, stop=True,
                        )

                for n, i in enumerate((0, 4, 1, 5, 2, 6, 3, 7)):
                    wt = wts[i]
                    ps = ppool.tile([128, CW], F32)
                    for k in range(KT):
                        nc.tensor.matmul(
                            ps[0:64, :], h3_sb[:, k, :], wt[:, 0, k, :],
                            start=(k == 0), stop=(k == KT - 1), tile_position=(0, 0),
                        )
                        nc.tensor.matmul(
                            ps[64:128, :], h3_sb[:, k, :], wt[:, 1, k, :],
                            start=(k == 0), stop=(k == KT - 1), tile_position=(0, 64),
                        )
                    # residual = psum - Pm/SP, written bf16
                    rt = rpool.tile([128, CW], BF16, tag="rt")
                    cb = i * CW
                    STT(out=rt[:, :], in0=Pm_sb[:, cb : cb + CW],
                        scalar=-1.0 / SP, in1=ps[:, :], op0=MUL, op1=ADD)
                    # out stores split 5/3 across rings to balance total bytes
                    oeng = dma if n < 5 else nc.scalar.dma_start
                    oeng(out=out[i], in_=rt[:, :])
                    if n < 7:
                        warm2(6 if n == 6 else 2)

    nc.compile()
    return nc


def _lap(x):
    # reflect-pad width-1 Laplacian on the last two axes (dx = dy = 1)
    p = np.pad(x, [(0, 0)] * (x.ndim - 2) + [(1, 1), (0, 0)], mode="reflect")
    d2y = p[..., :-2, :] - 2.0 * x + p[..., 2:, :]
    p = np.pad(x, [(0, 0)] * (x.ndim - 2) + [(0, 0), (1, 1)], mode="reflect")
    d2x = p[..., :-2] - 2.0 * x + p[..., 2:]
    return d2x + d2y


def make_in_maps(inputs):
    f32 = np.float32
    # offline weight prep: fold the stencil operator into W4/b4
    W4i = np.asarray(inputs["W4"], dtype=f32).reshape(1024, H, W)
    L1 = _lap(W4i)
    W4p = (_lap(L1) + L1 + W4i).reshape(1024, H * W)
    b4i = np.asarray(inputs["b4"], dtype=f32).reshape(H, W)
    l1 = _lap(b4i)
    b4p = (_lap(l1) + l1 + b4i).reshape(H * W)

    W4q_all = np.clip(W4p * SW, -15.5, 15.5).astype(FP8_NP)  # [1024, 65536]

    W2t = np.asarray(inputs["W2"], dtype=f32).reshape(2, 128, 512).transpose(1, 0, 2).reshape(128, 1024)
    # 1/SW folded into W3 (exact: power-of-two scale, relu-homogeneous)
    W3t = (np.asarray(inputs["W3"], dtype=f32) / SW).reshape(4, 128, 1024).transpose(1, 0, 2).reshape(128, 4096)
    shared = {
        "XW1": np.ascontiguousarray(
            np.concatenate([inputs["x_coloc"].T, inputs["W1"]], axis=1), dtype=f32
        ),
        "W2M": np.ascontiguousarray(W2t.astype(BF16_NP)),
        "W3Q": np.ascontiguousarray(W3t.reshape(128, 4, 1024).astype(BF16_NP)),
        "bias": np.ascontiguousarray(
            np.concatenate(
                [
                    np.asarray(inputs["b1"], dtype=f32).reshape(2, 128).T,
                    np.asarray(inputs["b2"], dtype=f32).reshape(4, 128).T,
                    (np.asarray(inputs["b3"], dtype=f32) / SW).reshape(8, 128).T,
                ],
                axis=1,
            )
        ),
    }

    Pme = (np.asarray(inputs["P"], dtype=f32) - b4p[None, :]) * SP  # [B, 65536]
    in_maps = []
    for c in range(NCORES):
        c0 = c * PIX
        # [kt, kp, half, cp, px] -> [kp, cp, half, kt, px]
        Wc = W4q_all[:, c0 : c0 + PIX].reshape(KT, 128, 2, CP, CW).transpose(1, 3, 2, 0, 4)
        Pc = Pme[:, c0 : c0 + PIX].reshape(B, 2, HALF)
        Pc = np.concatenate([Pc[:, 0, :], Pc[:, 1, :]], axis=0)  # [128, HALF]
        m = dict(shared)
        m["W4q"] = np.ascontiguousarray(Wc)
        m["Pm"] = np.clip(Pc, -15.5, 15.5).astype(FP8_NP)
        in_maps.append(m)
    return in_maps


def assemble_output(results):
    outf = np.empty((B, H * W), dtype=np.float32)
    for c in range(NCORES):
        oc = np.asarray(results[c]["out"])  # [CP, 128, CW] bf16
        # [cp, half*64+b, px] -> [b, half, cp, px]
        blk = oc.reshape(CP, 2, B, CW).transpose(2, 1, 0, 3).reshape(B, PIX)
        outf[:, c * PIX : (c + 1) * PIX] = blk.astype(np.float32)
    return outf


def get_program():
    if "nc" not in _PROGRAM_CACHE:
        _PROGRAM_CACHE["nc"] = _build_program()
    return _PROGRAM_CACHE["nc"]


def kernel(**inputs):
    nc = get_program()
    in_maps = make_in_maps(inputs)
    res = run_bass_kernel_spmd(nc, in_maps, list(range(NCORES)))
    return assemble_output(res.results)
